# revision 3
# baseline (speedup 1.0000x reference)
"""RAFT-style CorrBlock kernel for Trainium2 (8 NeuronCores, Bass/Tile).

Full inputs: fmap1 [2,256,64,64], fmap2 [2,256,64,64], centroids_coords [2,2,64,64].
Output: [2, 324, 64, 64] f32.

Sharding: data-parallel over the B*H1*W1 query-pixel axis, with pixels of each
batch SORTED by y0 = floor(ccy) and dealt round-robin to that batch's 4 cores.
Group g on every core then draws from the same sorted block of 512 pixels, so
its y0 range (hence the corr band it needs) is narrow and identical across
cores -- the band geometry is baked into the (single, SPMD) program.

Per core:
  - bf16 matmuls f1_chunk^T @ f2T_pad over only the banded rows each group can
    sample (pooling commutes with the channel contraction; the 4-level f2
    pyramid + zero padding is prepared on host, transposed to x-major so the
    corr slab comes out column-major),
  - one DRAM slab write per (group, level-0) and per (group, levels 1-3),
  - one indirect-DMA band gather per (group, level): 9*HB+10 contiguous bf16
    cover the whole 10x10 patch (column-major, padded => no masking),
  - bilinear combine with host-expanded per-pixel weights (all-bf16, packed),
  - host un-permutes rows and assembles the full output.
"""

import numpy as np
import ml_dtypes

import concourse.bass as bass
import concourse.bacc as bacc
import concourse.mybir as mybir
import concourse.tile as tile
from concourse.bass_utils import run_bass_kernel_spmd

f32 = mybir.dt.float32
bf16 = mybir.dt.bfloat16
i32 = mybir.dt.int32
BF = ml_dtypes.bfloat16
OP = mybir.AluOpType

P = 128
C = 256
NPIX = 1024        # query pixels per core
NG = NPIX // P     # 8 groups of 128 pixels
NLVL = 4
S = 9              # sample window side
PS = 10            # patch side
W_L = [64, 32, 16, 8]
WP_L = [w + 9 for w in W_L]            # padded width (x in [-4, w+4])
HP_L = [w + 9 for w in W_L]            # padded height (y in [-4, w+4])
F2OFF = [0]
for _l in range(1, NLVL):
    F2OFF.append(F2OFF[-1] + WP_L[_l - 1] * HP_L[_l - 1])
F2TOT = F2OFF[-1] + WP_L[-1] * HP_L[-1]   # 7924
FEAT = NLVL * S * S                    # 324


def _ap_view(t_ap, offset, dims):
    """Arbitrary strided view of a tile AP: dims = [[step, count], ...] free dims."""
    return bass.AP(t_ap.tensor, t_ap.offset + offset, [list(t_ap.ap[0])] + dims)


def make_plan(centroids_coords):
    """Host-side geometry: pixel->core permutation and per-group band layout.

    Returns a dict with all baked constants + per-core input arrays' indices.
    """
    cc = np.asarray(centroids_coords, dtype=np.float32)  # [2, 2, 64, 64]
    ccf = cc.reshape(2, 2, 4096)
    y0_all = np.floor(ccf[:, 1]).astype(np.int64)        # [2, 4096]
    x0_all = np.floor(ccf[:, 0]).astype(np.int64)

    perms = [np.argsort(y0_all[b], kind="stable") for b in range(2)]
    # core c (of batch b) takes sorted positions s with s % 4 == c
    core_pix = []
    for b in range(2):
        for c in range(4):
            core_pix.append(perms[b][c::4])              # [1024] original pixel ids

    rmin = np.full(NG, 64, dtype=np.int64)
    rmax = np.full(NG, -1, dtype=np.int64)
    for b in range(2):
        ys = y0_all[b][perms[b]]
        for g in range(NG):
            blk = ys[g * 512:(g + 1) * 512]
            rmin[g] = min(rmin[g], blk.min())
            rmax[g] = max(rmax[g], blk.max())

    HB = []
    starts = []   # [NLVL][NG], in padded-y coords
    for l in range(NLVL):
        hbl = int(max((rmax[g] >> l) - (rmin[g] >> l) for g in range(NG))) + 10
        hbl = min(hbl, HP_L[l])
        HB.append(hbl)
        starts.append([int(min(max(rmin[g] >> l, 0), HP_L[l] - hbl)) for g in range(NG)])

    F_l = [WP_L[l] * HB[l] for l in range(NLVL)]
    F0 = F_l[0]
    OFF123 = [0, F_l[1], F_l[1] + F_l[2]]
    F123 = F_l[1] + F_l[2] + F_l[3]
    BL = [9 * HB[l] + 10 for l in range(NLVL)]

    return dict(
        core_pix=core_pix, HB=HB, starts=starts, F0=F0, F123=F123,
        OFF123=OFF123, BL=BL, y0_all=y0_all, x0_all=x0_all, ccf=ccf,
    )


def build_bass(HB, starts, F0, F123, OFF123, BL):
    nc = bacc.Bacc("TRN2", target_bir_lowering=False, debug=False)

    f1_d = nc.dram_tensor("f1", [C, NPIX], bf16, kind="ExternalInput")
    f2a_d = nc.dram_tensor("f2a", [P, F2TOT], bf16, kind="ExternalInput")
    f2b_d = nc.dram_tensor("f2b", [P, F2TOT], bf16, kind="ExternalInput")
    idx_d = nc.dram_tensor("idx", [P, NG * NLVL], i32, kind="ExternalInput")
    wexp_d = nc.dram_tensor("wexp", [P, NLVL * 4 * NG * S * S], bf16, kind="ExternalInput")
    out_d = nc.dram_tensor("out", [NPIX, FEAT], bf16, kind="ExternalOutput")
    slab0_d = [nc.dram_tensor(f"slab0_{g}", [P * F0], bf16) for g in range(NG)]
    slab123_d = [nc.dram_tensor(f"slab123_{g}", [P * F123], bf16) for g in range(NG)]

    with tile.TileContext(nc) as tc:
        with (
            tc.tile_pool(name="persist", bufs=1) as pp,
            tc.tile_pool(name="grp", bufs=3) as pg,
            tc.tile_pool(name="psum", bufs=8, space="PSUM") as ps,
            tc.tile_pool(name="post", bufs=2) as po,
        ):
            # ---- input loads ----
            idx = pp.tile([P, NG * NLVL], i32, tag="idx")
            nc.sync.dma_start(idx[:], idx_d.ap())
            f1t = []
            for k in range(2):
                t1 = pp.tile([P, NPIX], bf16, tag=f"f1_{k}", name=f"f1_{k}")
                nc.sync.dma_start(t1[:], f1_d.ap()[k * P:(k + 1) * P, :])
                f1t.append(t1)
            f2t = []
            for k, src in enumerate((f2a_d, f2b_d)):
                t2 = pp.tile([P, F2TOT], bf16, tag=f"f2_{k}", name=f"f2_{k}")
                nc.sync.dma_start(t2[:], src.ap())
                f2t.append(t2)
            wexp = pp.tile([P, NLVL * 4 * NG * S * S], bf16, tag="wexp")
            nc.sync.dma_start(wexp[:], wexp_d.ap())

            band = [pp.tile([P, NG * BL[l]], bf16, tag=f"band_{l}", name=f"band_{l}")
                    for l in range(NLVL)]
            featsH = [pp.tile([P, 4 * FEAT], bf16, tag=f"feats_{h}", name=f"feats_{h}")
                      for h in range(2)]

            ncopy = 0

            def psum_chunks(g, l, corr_dst, dst_off):
                """matmul the (g, l) band in <=512-col PSUM chunks, copy to corr."""
                nonlocal ncopy
                hb = HB[l]
                cx = max(1, 512 // hb)
                x = 0
                while x < WP_L[l]:
                    w = min(cx, WP_L[l] - x)
                    pt = ps.tile([P, 512], f32, tag="mm", name=f"mm_{g}_{l}_{x}")
                    for k in range(2):
                        nc.tensor.matmul(
                            out=pt[:, :w * hb],
                            lhsT=f1t[k][:, g * P:(g + 1) * P],
                            rhs=_ap_view(
                                f2t[k][:],
                                F2OFF[l] + x * HP_L[l] + starts[l][g],
                                [[HP_L[l], w], [1, hb]],
                            ),
                            start=(k == 0),
                            stop=(k == 1),
                        )
                    dst = corr_dst[:, dst_off + x * hb: dst_off + (x + w) * hb]
                    if ncopy % 3 == 2:
                        nc.vector.tensor_copy(out=dst, in_=pt[:, :w * hb])
                    else:
                        nc.scalar.copy(out=dst, in_=pt[:, :w * hb])
                    ncopy += 1
                    x += w

            def combine_half(h):
                """Weighted 4-tap combine for groups [4h, 4h+4), all levels."""
                base = 4 * h
                tA = po.tile([P, 4 * S * S], bf16, tag="tA", name=f"tA_{h}")
                tB = po.tile([P, 4 * S * S], bf16, tag="tB", name=f"tB_{h}")
                tAv = tA[:].rearrange("p (g a b) -> p g a b", a=S, b=S)
                tBv = tB[:].rearrange("p (g a b) -> p g a b", a=S, b=S)
                for l in range(NLVL):
                    hb = HB[l]

                    def pv(a, b):
                        return _ap_view(
                            band[l][:], base * BL[l] + b * hb + a,
                            [[BL[l], 4], [hb, S], [1, S]],
                        )

                    def wv(ab):
                        return _ap_view(
                            wexp[:], ((l * 4 + ab) * NG + base) * S * S,
                            [[S * S, 4], [S, S], [1, S]],
                        )

                    ov = _ap_view(featsH[h][:], l * S * S, [[FEAT, 4], [S, S], [1, S]])
                    nc.vector.tensor_tensor(out=tAv, in0=pv(0, 0), in1=wv(0), op=OP.mult)
                    nc.vector.tensor_tensor(out=tBv, in0=pv(0, 1), in1=wv(1), op=OP.mult)
                    nc.vector.tensor_tensor(out=tAv, in0=tAv, in1=tBv, op=OP.add)
                    nc.vector.tensor_tensor(out=tBv, in0=pv(1, 0), in1=wv(2), op=OP.mult)
                    nc.vector.tensor_tensor(out=tAv, in0=tAv, in1=tBv, op=OP.add)
                    nc.vector.tensor_tensor(out=tBv, in0=pv(1, 1), in1=wv(3), op=OP.mult)
                    nc.vector.tensor_tensor(out=ov, in0=tAv, in1=tBv, op=OP.add)
                nc.sync.dma_start(
                    out_d.ap().rearrange("(g p) f -> p g f", p=P)[:, base:base + 4, :],
                    featsH[h][:].rearrange("p (g f) -> p g f", f=FEAT),
                )

            for g in range(NG):
                corr0 = pg.tile([P, F0], bf16, tag="corr0", name=f"corr0_{g}")
                psum_chunks(g, 0, corr0, 0)
                nc.sync.dma_start(
                    slab0_d[g].ap().rearrange("(p f) -> p f", f=F0), corr0[:])
                corr1 = pg.tile([P, F123], bf16, tag="corr123", name=f"corr123_{g}")
                for l in range(1, NLVL):
                    psum_chunks(g, l, corr1, OFF123[l - 1])
                nc.sync.dma_start(
                    slab123_d[g].ap().rearrange("(p f) -> p f", f=F123), corr1[:])
                for l in range(NLVL):
                    src = slab0_d[g] if l == 0 else slab123_d[g]
                    nc.gpsimd.indirect_dma_start(
                        out=band[l][:, g * BL[l]:(g + 1) * BL[l]],
                        out_offset=None,
                        in_=src.ap()[:, None],
                        in_offset=bass.IndirectOffsetOnAxis(
                            ap=idx[:, g * NLVL + l: g * NLVL + l + 1], axis=0),
                        element_offset=0,
                    )
                if g == 3:
                    combine_half(0)
            combine_half(1)

    nc.compile()
    return nc


_NC_CACHE = {}
LAST_PLAN = None


def _get_nc(plan):
    key = (tuple(plan["HB"]),
           tuple(tuple(s) for s in plan["starts"]))
    if key not in _NC_CACHE:
        _NC_CACHE[key] = build_bass(
            plan["HB"], plan["starts"], plan["F0"], plan["F123"],
            plan["OFF123"], plan["BL"])
    return _NC_CACHE[key]


def make_in_maps(fmap1, fmap2, centroids_coords, plan=None):
    global LAST_PLAN
    if plan is None:
        plan = make_plan(centroids_coords)
    LAST_PLAN = plan
    HB, starts, BL = plan["HB"], plan["starts"], plan["BL"]
    F0, F123, OFF123 = plan["F0"], plan["F123"], plan["OFF123"]

    fmap1 = np.asarray(fmap1, dtype=np.float32)
    fmap2 = np.asarray(fmap2, dtype=np.float32)

    # f2 pyramid, padded + transposed to x-major, per batch (shared by 4 cores)
    f2halves = []
    for b in range(2):
        pyr = fmap2[b]  # [C, 64, 64]
        full = np.zeros((C, F2TOT), dtype=np.float32)
        cur = pyr
        for l in range(NLVL):
            w = W_L[l]
            padded = np.zeros((C, WP_L[l], HP_L[l]), dtype=np.float32)
            padded[:, 4:4 + w, 4:4 + w] = cur.transpose(0, 2, 1)  # [c, x, y]
            full[:, F2OFF[l]:F2OFF[l] + WP_L[l] * HP_L[l]] = padded.reshape(C, -1)
            if l + 1 < NLVL:
                cur = cur.reshape(C, w // 2, 2, w // 2, 2).mean(axis=(2, 4))
        f2halves.append([np.ascontiguousarray(full[k * P:(k + 1) * P]).astype(BF)
                         for k in range(2)])

    in_maps = []
    for core in range(8):
        b = core // 4
        pix = plan["core_pix"][core]                      # [1024] original ids
        f1 = (fmap1[b].reshape(C, 4096)[:, pix] * (1.0 / 16.0)).astype(BF)

        ccx = plan["ccf"][b, 0, pix]                      # [1024] f32
        ccy = plan["ccf"][b, 1, pix]
        # slot k -> (g, p): g = k // 128, p = k % 128
        gi = np.arange(NPIX) // P
        pi = np.arange(NPIX) % P

        idx = np.zeros((P, NG * NLVL), dtype=np.int32)
        wexp = np.zeros((P, NLVL, 4, NG, S * S), dtype=np.float32)
        for l in range(NLVL):
            inv = 1.0 / (1 << l)
            xs = ccx * inv
            ys = ccy * inv
            x0 = np.floor(xs).astype(np.int64)
            y0 = np.floor(ys).astype(np.int64)
            fx = (xs - x0).astype(np.float32)
            fy = (ys - y0).astype(np.float32)
            st = np.asarray(starts[l], dtype=np.int64)[gi]
            assert (y0 >= st).all() and (y0 - st <= HB[l] - 10).all()
            assert (x0 >= 0).all() and (x0 <= W_L[l] - 1).all()
            base = OFF123[l - 1] if l > 0 else 0
            ftot = F123 if l > 0 else F0
            off = pi * ftot + base + x0 * HB[l] + (y0 - st)
            idx[pi, gi * NLVL + l] = off.astype(np.int32)
            for ab, (wa, wb) in enumerate(
                    (((1 - fy), (1 - fx)), ((1 - fy), fx), (fy, (1 - fx)), (fy, fx))):
                wexp[pi, l, ab, gi, :] = (wa * wb)[:, None]
        in_maps.append({
            "f1": f1,
            "f2a": f2halves[b][0], "f2b": f2halves[b][1],
            "idx": idx,
            "wexp": np.ascontiguousarray(wexp.reshape(P, -1)).astype(BF),
        })
    return in_maps


def assemble(outs, plan):
    """outs: list of 8 arrays [1024, 324] -> [2, 324, 64, 64] f32."""
    full = np.empty((2, FEAT, 64, 64), dtype=np.float32)
    for b in range(2):
        feats = np.empty((4096, FEAT), dtype=np.float32)
        for c in range(4):
            feats[plan["core_pix"][b * 4 + c]] = np.asarray(
                outs[b * 4 + c], dtype=np.float32)
        full[b] = feats.reshape(64, 64, FEAT).transpose(2, 0, 1)
    return full


def kernel(fmap1, fmap2, centroids_coords, trace=False):
    plan = make_plan(centroids_coords)
    nc = _get_nc(plan)
    in_maps = make_in_maps(fmap1, fmap2, centroids_coords, plan)
    try:
        res = run_bass_kernel_spmd(nc, in_maps, core_ids=list(range(8)), trace=trace)
    except ModuleNotFoundError:
        res = run_bass_kernel_spmd(nc, in_maps, core_ids=list(range(8)), trace=False)
    out = assemble([r["out"] for r in res.results], plan)
    if trace:
        kernel.last_result = res
    return out


# revision 6
# speedup vs baseline: 1.0381x; 1.0381x over previous
"""RAFT-style CorrBlock kernel for Trainium2 (8 NeuronCores, Bass/Tile).

Full inputs: fmap1 [2,256,64,64], fmap2 [2,256,64,64], centroids_coords [2,2,64,64].
Output: [2, 324, 64, 64] f32.

Sharding: data-parallel over the B*H1*W1 query-pixel axis, with pixels of each
batch SORTED by y0 = floor(ccy) and dealt round-robin to that batch's 4 cores.
Group g on every core then draws from the same sorted block of 512 pixels, so
its y0 range (hence the corr band it needs) is narrow and identical across
cores -- the band geometry is baked into the (single, SPMD) program.

Per core:
  - bf16 matmuls f1_chunk^T @ f2T_pad over only the banded rows each group can
    sample (pooling commutes with the channel contraction; the 4-level f2
    pyramid + zero padding is prepared on host, transposed to x-major so the
    corr slab comes out column-major),
  - one DRAM slab write per (group, level-0) and per (group, levels 1-3),
  - one indirect-DMA band gather per (group, level): 9*HB+10 contiguous bf16
    cover the whole 10x10 patch (column-major, padded => no masking),
  - bilinear combine with host-expanded per-pixel weights (all-bf16, packed),
  - host un-permutes rows and assembles the full output.
"""

import numpy as np
import ml_dtypes

import concourse.bass as bass
import concourse.bacc as bacc
import concourse.mybir as mybir
import concourse.tile as tile
from concourse.bass_utils import run_bass_kernel_spmd

f32 = mybir.dt.float32
bf16 = mybir.dt.bfloat16
i32 = mybir.dt.int32
BF = ml_dtypes.bfloat16
OP = mybir.AluOpType

P = 128
C = 256
NPIX = 1024        # query pixels per core
NG = NPIX // P     # 8 groups of 128 pixels
NLVL = 4
S = 9              # sample window side
PS = 10            # patch side
W_L = [64, 32, 16, 8]
WP_L = [w + 9 for w in W_L]            # padded width (x in [-4, w+4])
HP_L = [w + 9 for w in W_L]            # padded height (y in [-4, w+4])
F2OFF = [0]
for _l in range(1, NLVL):
    F2OFF.append(F2OFF[-1] + WP_L[_l - 1] * HP_L[_l - 1])
F2TOT = F2OFF[-1] + WP_L[-1] * HP_L[-1]   # 7924
FEAT = NLVL * S * S                    # 324


def _ap_view(t_ap, offset, dims):
    """Arbitrary strided view of a tile AP: dims = [[step, count], ...] free dims."""
    return bass.AP(t_ap.tensor, t_ap.offset + offset, [list(t_ap.ap[0])] + dims)


def make_plan(centroids_coords):
    """Host-side geometry: pixel->core permutation and per-group band layout.

    Returns a dict with all baked constants + per-core input arrays' indices.
    """
    cc = np.asarray(centroids_coords, dtype=np.float32)  # [2, 2, 64, 64]
    ccf = cc.reshape(2, 2, 4096)
    y0_all = np.floor(ccf[:, 1]).astype(np.int64)        # [2, 4096]
    x0_all = np.floor(ccf[:, 0]).astype(np.int64)

    perms = [np.argsort(y0_all[b], kind="stable") for b in range(2)]
    # core c (of batch b) takes sorted positions s with s % 4 == c
    core_pix = []
    for b in range(2):
        for c in range(4):
            core_pix.append(perms[b][c::4])              # [1024] original pixel ids

    rmin = np.full(NG, 64, dtype=np.int64)
    rmax = np.full(NG, -1, dtype=np.int64)
    for b in range(2):
        ys = y0_all[b][perms[b]]
        for g in range(NG):
            blk = ys[g * 512:(g + 1) * 512]
            rmin[g] = min(rmin[g], blk.min())
            rmax[g] = max(rmax[g], blk.max())

    HB = []
    starts = []   # [NLVL][NG], in padded-y coords
    for l in range(NLVL):
        hbl = int(max((rmax[g] >> l) - (rmin[g] >> l) for g in range(NG))) + 10
        hbl = min(hbl, HP_L[l])
        HB.append(hbl)
        starts.append([int(min(max(rmin[g] >> l, 0), HP_L[l] - hbl)) for g in range(NG)])

    F_l = [WP_L[l] * HB[l] for l in range(NLVL)]
    F0 = F_l[0]
    OFF123 = [0, F_l[1], F_l[1] + F_l[2]]
    F123 = F_l[1] + F_l[2] + F_l[3]
    BL = [9 * HB[l] + 10 for l in range(NLVL)]

    return dict(
        core_pix=core_pix, HB=HB, starts=starts, F0=F0, F123=F123,
        OFF123=OFF123, BL=BL, y0_all=y0_all, x0_all=x0_all, ccf=ccf,
    )


def build_bass(HB, starts, F0, F123, OFF123, BL):
    nc = bacc.Bacc("TRN2", target_bir_lowering=False, debug=False)

    F20 = WP_L[0] * HP_L[0]
    F21 = F2TOT - F20

    f1_d = nc.dram_tensor("f1", [C, NPIX], bf16, kind="ExternalInput")
    f2l0_d = [nc.dram_tensor(f"f2l0_{k}", [P, F20], bf16, kind="ExternalInput")
              for k in range(2)]
    f2l1_d = [nc.dram_tensor(f"f2l1_{k}", [P, F21], bf16, kind="ExternalInput")
              for k in range(2)]
    idx_d = nc.dram_tensor("idx", [P, NG * NLVL], i32, kind="ExternalInput")
    wexp_d = nc.dram_tensor("wexp", [P, NLVL * 4 * NG * S * S], bf16, kind="ExternalInput")
    out_d = nc.dram_tensor("out", [NPIX, FEAT], bf16, kind="ExternalOutput")
    slab0_d = [nc.dram_tensor(f"slab0_{g}", [P * F0], bf16) for g in range(NG)]
    slab123_d = [nc.dram_tensor(f"slab123_{g}", [P * F123], bf16) for g in range(NG)]

    with tile.TileContext(nc) as tc:
        with (
            tc.tile_pool(name="persist", bufs=1) as pp,
            tc.tile_pool(name="grp", bufs=3) as pg,
            tc.tile_pool(name="psum", bufs=8, space="PSUM") as ps,
            tc.tile_pool(name="post", bufs=2) as po,
        ):
            # ---- input loads (ordered by when the pipeline needs them) ----
            idx = pp.tile([P, NG * NLVL], i32, tag="idx")
            nc.sync.dma_start(idx[:], idx_d.ap())
            f1t = []
            for k in range(2):
                t1 = pp.tile([P, NPIX], bf16, tag=f"f1_{k}", name=f"f1_{k}")
                nc.sync.dma_start(t1[:], f1_d.ap()[k * P:(k + 1) * P, :])
                f1t.append(t1)
            f2t0 = []
            for k in range(2):
                t2 = pp.tile([P, F20], bf16, tag=f"f2l0_{k}", name=f"f2l0_{k}")
                nc.sync.dma_start(t2[:], f2l0_d[k].ap())
                f2t0.append(t2)
            f2t1 = []
            for k in range(2):
                t2 = pp.tile([P, F21], bf16, tag=f"f2l1_{k}", name=f"f2l1_{k}")
                nc.sync.dma_start(t2[:], f2l1_d[k].ap())
                f2t1.append(t2)
            wexp = pp.tile([P, NLVL * 4 * NG * S * S], bf16, tag="wexp")
            nc.sync.dma_start(wexp[:], wexp_d.ap())

            band = [pp.tile([P, NG * BL[l]], bf16, tag=f"band_{l}", name=f"band_{l}")
                    for l in range(NLVL)]
            featsH = [pp.tile([P, 4 * FEAT], bf16, tag=f"feats_{h}", name=f"feats_{h}")
                      for h in range(2)]

            ncopy = 0

            def psum_chunks(g, l, corr_dst, dst_off):
                """matmul the (g, l) band in <=512-col PSUM chunks, copy to corr."""
                nonlocal ncopy
                hb = HB[l]
                cx = max(1, 512 // hb)
                f2t = f2t0 if l == 0 else f2t1
                f2off = 0 if l == 0 else F2OFF[l] - F2OFF[1]
                x = 0
                while x < WP_L[l]:
                    w = min(cx, WP_L[l] - x)
                    pt = ps.tile([P, 512], f32, tag="mm", name=f"mm_{g}_{l}_{x}")
                    for k in range(2):
                        nc.tensor.matmul(
                            out=pt[:, :w * hb],
                            lhsT=f1t[k][:, g * P:(g + 1) * P],
                            rhs=_ap_view(
                                f2t[k][:],
                                f2off + x * HP_L[l] + starts[l][g],
                                [[HP_L[l], w], [1, hb]],
                            ),
                            start=(k == 0),
                            stop=(k == 1),
                        )
                    dst = corr_dst[:, dst_off + x * hb: dst_off + (x + w) * hb]
                    if ncopy % 3 == 2:
                        nc.vector.tensor_copy(out=dst, in_=pt[:, :w * hb])
                    else:
                        nc.scalar.copy(out=dst, in_=pt[:, :w * hb])
                    ncopy += 1
                    x += w

            def gather(g, l):
                src = slab0_d[g] if l == 0 else slab123_d[g]
                nc.gpsimd.indirect_dma_start(
                    out=band[l][:, g * BL[l]:(g + 1) * BL[l]],
                    out_offset=None,
                    in_=src.ap()[:, None],
                    in_offset=bass.IndirectOffsetOnAxis(
                        ap=idx[:, g * NLVL + l: g * NLVL + l + 1], axis=0),
                    element_offset=0,
                )

            def combine(base, n, l):
                """Weighted 4-tap combine for groups [base, base+n), level l."""
                hb = HB[l]
                tA = po.tile([P, 4 * S * S], bf16, tag="tA", name=f"tA_{base}_{l}")
                tB = po.tile([P, 4 * S * S], bf16, tag="tB", name=f"tB_{base}_{l}")
                tAv = tA[:, :n * S * S].rearrange("p (g a b) -> p g a b", a=S, b=S)
                tBv = tB[:, :n * S * S].rearrange("p (g a b) -> p g a b", a=S, b=S)

                def pv(a, b):
                    return _ap_view(
                        band[l][:], base * BL[l] + b * hb + a,
                        [[BL[l], n], [hb, S], [1, S]],
                    )

                def wv(ab):
                    return _ap_view(
                        wexp[:], ((l * 4 + ab) * NG + base) * S * S,
                        [[S * S, n], [S, S], [1, S]],
                    )

                h = base // 4
                ov = _ap_view(featsH[h][:], (base - 4 * h) * FEAT + l * S * S,
                              [[FEAT, n], [S, S], [1, S]])
                nc.vector.tensor_tensor(out=tAv, in0=pv(0, 0), in1=wv(0), op=OP.mult)
                nc.vector.tensor_tensor(out=tBv, in0=pv(0, 1), in1=wv(1), op=OP.mult)
                nc.vector.tensor_tensor(out=tAv, in0=tAv, in1=tBv, op=OP.add)
                nc.vector.tensor_tensor(out=tBv, in0=pv(1, 0), in1=wv(2), op=OP.mult)
                nc.vector.tensor_tensor(out=tAv, in0=tAv, in1=tBv, op=OP.add)
                nc.vector.tensor_tensor(out=tBv, in0=pv(1, 1), in1=wv(3), op=OP.mult)
                nc.vector.tensor_tensor(out=ov, in0=tAv, in1=tBv, op=OP.add)

            def ship_half(h):
                nc.sync.dma_start(
                    out_d.ap().rearrange("(g p) f -> p g f", p=P)[:, 4 * h:4 * h + 4, :],
                    featsH[h][:].rearrange("p (g f) -> p g f", f=FEAT),
                )

            # ---- phase A: level 0 for all groups (only needs f2 L0) ----
            for g in range(NG):
                corr0 = pg.tile([P, F0], bf16, tag="corr0", name=f"corr0_{g}")
                psum_chunks(g, 0, corr0, 0)
                nc.sync.dma_start(
                    slab0_d[g].ap().rearrange("(p f) -> p f", f=F0), corr0[:])
                gather(g, 0)
                if g == 3:
                    combine(0, 4, 0)
                elif g == 7:
                    combine(4, 4, 0)

            # ---- phase B: levels 1-3 for all groups ----
            for g in range(NG):
                corr1 = pg.tile([P, F123], bf16, tag="corr123", name=f"corr123_{g}")
                for l in range(1, NLVL):
                    psum_chunks(g, l, corr1, OFF123[l - 1])
                nc.sync.dma_start(
                    slab123_d[g].ap().rearrange("(p f) -> p f", f=F123), corr1[:])
                for l in range(1, NLVL):
                    gather(g, l)
                if g == 3:
                    for l in range(1, NLVL):
                        combine(0, 4, l)
                    ship_half(0)
                elif g == 5:
                    for l in range(1, NLVL):
                        combine(4, 2, l)
                elif g == 7:
                    for l in range(1, NLVL):
                        combine(6, 2, l)
                    ship_half(1)

    nc.compile()
    return nc


_NC_CACHE = {}
LAST_PLAN = None


def _get_nc(plan):
    key = (tuple(plan["HB"]),
           tuple(tuple(s) for s in plan["starts"]))
    if key not in _NC_CACHE:
        _NC_CACHE[key] = build_bass(
            plan["HB"], plan["starts"], plan["F0"], plan["F123"],
            plan["OFF123"], plan["BL"])
    return _NC_CACHE[key]


def make_in_maps(fmap1, fmap2, centroids_coords, plan=None):
    global LAST_PLAN
    if plan is None:
        plan = make_plan(centroids_coords)
    LAST_PLAN = plan
    HB, starts, BL = plan["HB"], plan["starts"], plan["BL"]
    F0, F123, OFF123 = plan["F0"], plan["F123"], plan["OFF123"]

    fmap1 = np.asarray(fmap1, dtype=np.float32)
    fmap2 = np.asarray(fmap2, dtype=np.float32)

    # f2 pyramid, padded + transposed to x-major, per batch (shared by 4 cores)
    f2halves = []
    for b in range(2):
        pyr = fmap2[b]  # [C, 64, 64]
        full = np.zeros((C, F2TOT), dtype=np.float32)
        cur = pyr
        for l in range(NLVL):
            w = W_L[l]
            padded = np.zeros((C, WP_L[l], HP_L[l]), dtype=np.float32)
            padded[:, 4:4 + w, 4:4 + w] = cur.transpose(0, 2, 1)  # [c, x, y]
            full[:, F2OFF[l]:F2OFF[l] + WP_L[l] * HP_L[l]] = padded.reshape(C, -1)
            if l + 1 < NLVL:
                cur = cur.reshape(C, w // 2, 2, w // 2, 2).mean(axis=(2, 4))
        F20 = WP_L[0] * HP_L[0]
        f2halves.append([
            [np.ascontiguousarray(full[k * P:(k + 1) * P, :F20]).astype(BF)
             for k in range(2)],
            [np.ascontiguousarray(full[k * P:(k + 1) * P, F20:]).astype(BF)
             for k in range(2)],
        ])

    in_maps = []
    for core in range(8):
        b = core // 4
        pix = plan["core_pix"][core]                      # [1024] original ids
        f1 = (fmap1[b].reshape(C, 4096)[:, pix] * (1.0 / 16.0)).astype(BF)

        ccx = plan["ccf"][b, 0, pix]                      # [1024] f32
        ccy = plan["ccf"][b, 1, pix]
        # slot k -> (g, p): g = k // 128, p = k % 128
        gi = np.arange(NPIX) // P
        pi = np.arange(NPIX) % P

        idx = np.zeros((P, NG * NLVL), dtype=np.int32)
        wexp = np.zeros((P, NLVL, 4, NG, S * S), dtype=np.float32)
        for l in range(NLVL):
            inv = 1.0 / (1 << l)
            xs = ccx * inv
            ys = ccy * inv
            x0 = np.floor(xs).astype(np.int64)
            y0 = np.floor(ys).astype(np.int64)
            fx = (xs - x0).astype(np.float32)
            fy = (ys - y0).astype(np.float32)
            st = np.asarray(starts[l], dtype=np.int64)[gi]
            assert (y0 >= st).all() and (y0 - st <= HB[l] - 10).all()
            assert (x0 >= 0).all() and (x0 <= W_L[l] - 1).all()
            base = OFF123[l - 1] if l > 0 else 0
            ftot = F123 if l > 0 else F0
            off = pi * ftot + base + x0 * HB[l] + (y0 - st)
            idx[pi, gi * NLVL + l] = off.astype(np.int32)
            for ab, (wa, wb) in enumerate(
                    (((1 - fy), (1 - fx)), ((1 - fy), fx), (fy, (1 - fx)), (fy, fx))):
                wexp[pi, l, ab, gi, :] = (wa * wb)[:, None]
        in_maps.append({
            "f1": f1,
            "f2l0_0": f2halves[b][0][0], "f2l0_1": f2halves[b][0][1],
            "f2l1_0": f2halves[b][1][0], "f2l1_1": f2halves[b][1][1],
            "idx": idx,
            "wexp": np.ascontiguousarray(wexp.reshape(P, -1)).astype(BF),
        })
    return in_maps


def assemble(outs, plan):
    """outs: list of 8 arrays [1024, 324] -> [2, 324, 64, 64] f32."""
    full = np.empty((2, FEAT, 64, 64), dtype=np.float32)
    for b in range(2):
        feats = np.empty((4096, FEAT), dtype=np.float32)
        for c in range(4):
            feats[plan["core_pix"][b * 4 + c]] = np.asarray(
                outs[b * 4 + c], dtype=np.float32)
        full[b] = feats.reshape(64, 64, FEAT).transpose(2, 0, 1)
    return full


def kernel(fmap1, fmap2, centroids_coords, trace=False):
    plan = make_plan(centroids_coords)
    nc = _get_nc(plan)
    in_maps = make_in_maps(fmap1, fmap2, centroids_coords, plan)
    try:
        res = run_bass_kernel_spmd(nc, in_maps, core_ids=list(range(8)), trace=trace)
    except ModuleNotFoundError:
        res = run_bass_kernel_spmd(nc, in_maps, core_ids=list(range(8)), trace=False)
    out = assemble([r["out"] for r in res.results], plan)
    if trace:
        kernel.last_result = res
    return out
